# revision 22
# baseline (speedup 1.0000x reference)
"""DeepseekV4 Mega-MoE experts layer on 8 Trainium2 NeuronCores.

Strategy (expert-parallel, per sharding hint):
  - 16 experts sharded 2-per-core across 8 cores; each core receives its two
    experts' weights (losslessly converted: mxfp4*ue8m0 dequant values are
    exactly representable in TRN fp8_e4m3 for both w13 and w2).
  - Staging fp8 quantization of hidden_states is computed on the host exactly
    as the reference (per-32-group amax, UE8M0 ceil scale, fp8e4m3fn round
    trip), then cast to TRN fp8e4 — bit-exact except deep subnormals.
  - Token routing (the "all-to-all") happens on the host: per expert, the
    routed tokens' quantized activations are gathered transposed into xgT so
    the device only runs dense per-expert GEMMs.  Host sums the per-expert
    outputs (the "combine").

Per-core device pipeline (e = 2 local experts, cap tokens each):
  mm1: h[tok,1536] = xgT.T @ w13T, fp8 DoubleRow accumulating over d
       (w13 f-columns pre-permuted into paired [gate256|up256] blocks so each
       512-wide PSUM tile holds a gate/up pair -> 1 bank granularity)
  act: a = silu(gate) * comb * up        (ACT Silu + DVE STT, per 256-block)
  aT = PE transpose of a
  mm2: ye[tok,2048] = aT.T @ w2T, bf16 x fp8, accumulating over i
  DMA ye (bf16) out; host scatter-adds into [512,2048] fp32.

DMA supply order is matched to PE consumption order (xg/w13 chunks
u-interleaved, then w2) so the tensor engine never waits on weights.
"""

import sys

if "/opt/trn_rl_repo" not in sys.path:
    sys.path.insert(0, "/opt/trn_rl_repo")

import numpy as np
import ml_dtypes

T, D, I, E, TOPK, GROUP = 512, 2048, 768, 16, 8, 32
N_CORES = 8
E_LOC = E // N_CORES  # experts per core
DT, IT, FB, U = D // 128, I // 128, 3, D // 256  # 16, 6, 3, 8

FP8 = ml_dtypes.float8_e4m3      # TRN FP8_EXP4 (max 240) == bass dt.float8e4
BF16 = ml_dtypes.bfloat16

_FP4_TABLE = np.array(
    [0.0, 0.5, 1.0, 1.5, 2.0, 3.0, 4.0, 6.0,
     -0.0, -0.5, -1.0, -1.5, -2.0, -3.0, -4.0, -6.0], dtype=np.float32)

# f-permutation pairing gate/up 256-blocks: [g0|u0|g1|u1|g2|u2]
_FPERM = np.concatenate(
    [np.r_[256 * g:256 * (g + 1), I + 256 * g:I + 256 * (g + 1)]
     for g in range(FB)])


def _dequant_mxfp4(w_packed, sf):
    lo = _FP4_TABLE[w_packed & 0xF]
    hi = _FP4_TABLE[(w_packed >> 4) & 0xF]
    w = np.stack([lo, hi], axis=-1).reshape(*w_packed.shape[:-1], -1)
    s = (sf.astype(np.uint32) << 23).view(np.float32)
    w = w.reshape(*sf.shape, GROUP) * s[..., None]
    return w.reshape(*w_packed.shape[:-1], 2 * w_packed.shape[-1])


def _quant_dequant_fp8(x):
    """Exact replica of the reference staging quantization (host side)."""
    xg = x.reshape(T, D // GROUP, GROUP)
    amax = np.maximum(np.max(np.abs(xg), axis=-1), 1e-4).astype(np.float32)
    scale = (amax / np.float32(448.0)).astype(np.float32)
    bits = scale.view(np.uint32)
    exp = ((bits >> 23) & 0xFF) + ((bits & 0x7FFFFF) != 0).astype(np.uint32)
    exp = np.clip(exp, 1, 254).astype(np.uint32)
    rscale = (exp << 23).view(np.float32)
    q = (xg * (1.0 / rscale)[..., None]).astype(ml_dtypes.float8_e4m3fn)
    return (q.astype(np.float32) * rscale[..., None]).reshape(T, D)


_PROGRAM_CACHE = {}


def _build_program(cap, split_waits=True):
    import concourse.bass as bass
    import concourse.mybir as mybir
    import concourse.tile as tile
    from concourse.masks import make_identity

    _TC = tile.TileContext

    def _split_excess_waits(nc):
        # This walrus build accepts only ONE sem-wait per instruction; hoist
        # extra waits onto standalone EventSemaphore (pure-wait) instructions
        # on the same engine, which execute in order ahead of the original.
        n = 0
        for f in nc.m.functions:
            for b in f.blocks:
                out = []
                for ins in b.instructions:
                    si = ins.sync_info
                    waits = list(si.on_wait) if (si and si.on_wait) else []
                    if len(waits) > 1:
                        for k, w in enumerate(waits[:-1]):
                            out.append(mybir.InstEventSemaphore(
                                name=f"{ins.name}-xw{k}", engine=ins.engine,
                                ins=[], outs=[],
                                sync_info=mybir.SyncInfo(
                                    on_wait=[w], on_update=[])))
                            n += 1
                        si.on_wait = waits[-1:]
                    out.append(ins)
                b.instructions = out
        return n

    dt = mybir.dt
    MT = cap // 128            # token tiles per expert
    ECAP = E_LOC * cap

    nc = bass.Bass()
    xg_d = nc.dram_tensor("xg", [E_LOC, U, 128, 2, cap], dt.float8e4, kind="ExternalInput")
    w13_d = nc.dram_tensor("w13t", [E_LOC, DT, 128, 2 * I], dt.float8e4, kind="ExternalInput")
    w2_d = nc.dram_tensor("w2t", [E_LOC, IT, 128, D], dt.float8e4, kind="ExternalInput")
    comb_d = nc.dram_tensor("comb", [E_LOC, MT, 128, 1], dt.float32, kind="ExternalInput")
    ye_d = nc.dram_tensor("ye", [E_LOC, cap, D], dt.bfloat16, kind="ExternalOutput")

    with _TC(nc) as tc:
        with (
            tc.tile_pool(name="const", bufs=1) as constp,
            tc.tile_pool(name="wts", bufs=1) as wtsp,
            tc.tile_pool(name="sa", bufs=3) as sap,
            tc.tile_pool(name="a", bufs=2 * (cap // 128) * FB) as ap_,
            tc.tile_pool(name="yout", bufs=1) as youtp,
            tc.tile_pool(name="ps_h", bufs=6, space="PSUM") as psh,
            tc.tile_pool(name="ps_s", bufs=2, space="PSUM") as pss,
        ):
            # ---- input DMAs first, in PE-consumption order ----
            # xg on the scalar HW queue, w13/w2 on the sync HW queue (the
            # two hardware-dynamic rings), comb on gpsimd (tiny).
            xg = [[wtsp.tile([128, 2, cap], dt.float8e4, tag=f"xg_{e}_{u}",
                             name=f"xg_{e}_{u}") for u in range(U)]
                  for e in range(E_LOC)]
            w13 = [[wtsp.tile([128, 2, 2 * I], dt.float8e4, tag=f"w13_{e}_{u}", name=f"w13_{e}_{u}")
                    for u in range(U)] for e in range(E_LOC)]
            w2 = [[wtsp.tile([128, IT // 2, D], dt.float8e4, tag=f"w2_{e}_{h}", name=f"w2_{e}_{h}")
                   for h in range(2)] for e in range(E_LOC)]
            for e in range(E_LOC):
                for u in range(U):
                    nc.scalar.dma_start(xg[e][u][:], xg_d[e, u])
            for e in range(E_LOC):
                for u in range(U):
                    nc.sync.dma_start(
                        w13[e][u][:],
                        w13_d[e, 2 * u:2 * u + 2].rearrange("j p f -> p j f"))
            for e in range(E_LOC):
                for h in range(2):
                    k0 = h * (IT // 2)
                    nc.sync.dma_start(
                        w2[e][h][:],
                        w2_d[e, k0:k0 + IT // 2].rearrange("k p f -> p k f"))

            ident = constp.tile([128, 128], dt.bfloat16)
            make_identity(nc, ident[:])
            # warm the ACT Silu table before the first real silu
            warm = constp.tile([128, 8], dt.float32, tag="warm")
            nc.scalar.activation(
                warm[:], ident[:, 0:8], mybir.ActivationFunctionType.Silu)

            combg = []
            for e in range(E_LOC):
                cg = constp.tile([128, MT, 1], dt.float32, tag=f"cg_{e}",
                                 name=f"cg_{e}")
                nc.gpsimd.dma_start(cg[:], comb_d[e].rearrange("m p f -> p m f"))
                combg.append(cg)

            # per-k aT tiles so mm2's early k-ops don't wait on late copies
            aT = [[wtsp.tile([128, cap], dt.bfloat16, tag=f"aT_{e}_{k}",
                             name=f"aT_{e}_{k}") for k in range(IT)]
                  for e in range(E_LOC)]
            yes = [[youtp.tile([128, D], dt.bfloat16, tag=f"ye_{e}_{m}", name=f"ye_{e}_{m}")
                    for m in range(MT)] for e in range(E_LOC)]

            # PE warm-up: full-duty 512-free matmuls through the initial
            # DMA wait so the DVFS ramp completes before the first real op.
            wz = constp.tile([128, 512], dt.bfloat16, tag="wz")
            nc.vector.memset(wz[:], 0.0)
            wps = pss.tile([128, 512], dt.float32, tag="sm", name="warm_t")
            for _ in range(16):
                nc.tensor.matmul(wps[:], ident[:], wz[:], start=True, stop=True)
            # consume wps so its PSUM slot is released for mm2's groups
            nc.vector.tensor_copy(wz[:], wps[:])

            def mm1_and_silu(e):
                # h[tok, f] accumulated over d in DoubleRow ops; each
                # (m, fb) PSUM tile is one [128, gate256|up256] bank.
                # u is the outer loop so each w13 chunk is consumed at the
                # rate the DMA stream delivers it.
                out = []
                for mc in range(0, MT, 2):
                    ms = range(mc, min(mc + 2, MT))
                    hs = {m: [psh.tile([128, 512], dt.float32, tag="h",
                                       name=f"h_{e}_{m}_{fb}")
                              for fb in range(FB)] for m in ms}
                    for u in range(U):
                        for m in ms:
                            stat = xg[e][u][:, :, m * 128:(m + 1) * 128]
                            for fb in range(FB):
                                nc.tensor.matmul(
                                    hs[m][fb][:],
                                    stat,
                                    w13[e][u][:, :, fb * 512:(fb + 1) * 512],
                                    start=(u == 0), stop=(u == U - 1),
                                    perf_mode=mybir.MatmulPerfMode.DoubleRow)
                    for m in ms:
                        for fb in range(FB):
                            h = hs[m][fb]
                            s = sap.tile([128, 256], dt.float32, tag="silu")
                            nc.scalar.activation(
                                s[:], h[:, 0:256],
                                mybir.ActivationFunctionType.Silu)
                            a = ap_.tile([128, 256], dt.bfloat16, tag="a",
                                         name=f"a_{e}_{m}_{fb}")
                            # a = (silu(gate) * comb) * up
                            nc.vector.scalar_tensor_tensor(
                                a[:], s[:], combg[e][:, m, :], h[:, 256:512],
                                op0=mybir.AluOpType.mult,
                                op1=mybir.AluOpType.mult)
                            out.append((m, fb, a))
                return out

            def transposes(e, alist):
                # k-major so aT[e][k] tiles complete in mm2's consumption
                # order; copies alternate vector/scalar to halve the drain.
                byk = sorted(alist, key=lambda t: t[1])
                n = 0
                for m, fb, a in byk:
                    for half in range(2):
                        k = 2 * fb + half
                        pt = psh.tile([128, 128], dt.bfloat16, tag="h")
                        nc.tensor.transpose(
                            pt[:], a[:, half * 128:(half + 1) * 128], ident[:])
                        if n % 2 == 0:
                            nc.vector.tensor_copy(
                                aT[e][k][:, m * 128:(m + 1) * 128], pt[:])
                        else:
                            nc.scalar.copy(
                                aT[e][k][:, m * 128:(m + 1) * 128], pt[:])
                        n += 1

            def mm2(e):
                for m in range(MT):
                    for dq in range(4):
                        yh = pss.tile([128, 512], dt.float32, tag="sm")
                        for k in range(IT):
                            nc.tensor.matmul(
                                yh[:],
                                aT[e][k][:, m * 128:(m + 1) * 128],
                                w2[e][k // 3][:, k % 3, dq * 512:(dq + 1) * 512],
                                start=(k == 0), stop=(k == IT - 1))
                        nc.scalar.copy(
                            yes[e][m][:, dq * 512:(dq + 1) * 512], yh[:])
                        if dq % 2 == 1:
                            nc.sync.dma_start(
                                ye_d[e].rearrange("(m p) f -> p m f", p=128)
                                [:, m, (dq - 1) * 512:(dq + 1) * 512],
                                yes[e][m][:, (dq - 1) * 512:(dq + 1) * 512])

            # PE order: mm1(e0), mm1(e1), transp(e0), transp(e1), mm2(e0),
            # mm2(e1) — e1's aT copies drain lazily during mm2(e0)
            a0 = mm1_and_silu(0)
            a1 = mm1_and_silu(1)
            transposes(0, a0)
            transposes(1, a1)
            mm2(0)
            mm2(1)

    nc.finalize()
    if split_waits:
        _split_excess_waits(nc)
    return nc


def kernel(hidden_states, topk_weights, topk_ids, w13_weight, w13_weight_scale,
           w2_weight, w2_weight_scale):
    from concourse.bass_utils import run_bass_kernel_spmd

    x = np.ascontiguousarray(hidden_states, dtype=np.float32)
    tw = np.asarray(topk_weights, dtype=np.float32)
    ti = np.asarray(topk_ids)

    # host routing: combine weights + per-expert token lists
    comb = np.zeros((T, E), np.float32)
    for k in range(TOPK):
        np.add.at(comb, (np.arange(T), ti[:, k]), tw[:, k])
    routed = comb > 0.0
    idx = [np.nonzero(routed[:, e])[0] for e in range(E)]
    counts = [len(ix) for ix in idx]
    cap = max(128, -(-max(counts) // 128) * 128)

    if cap not in _PROGRAM_CACHE:
        _PROGRAM_CACHE[cap] = _build_program(cap)
    nc = _PROGRAM_CACHE[cap]

    # staging quantization (exact reference replica) + lossless weight dequant
    x8T = _quant_dequant_fp8(x).astype(FP8).T  # [D, T]
    w13 = _dequant_mxfp4(np.asarray(w13_weight), np.asarray(w13_weight_scale))
    w2 = _dequant_mxfp4(np.asarray(w2_weight), np.asarray(w2_weight_scale))
    MT = cap // 128

    in_maps = []
    for core in range(N_CORES):
        xgT = np.zeros((D, E_LOC * cap), FP8)
        cg = np.zeros((E_LOC, cap), np.float32)
        w13t = np.zeros((E_LOC, DT, 128, 2 * I), FP8)
        w2t = np.zeros((E_LOC, IT, 128, D), FP8)
        for le in range(E_LOC):
            e = core * E_LOC + le
            ix = idx[e]
            xgT[:, le * cap:le * cap + len(ix)] = x8T[:, ix]
            cg[le, :len(ix)] = comb[ix, e]
            w13t[le] = w13[e][_FPERM].T.astype(FP8).reshape(DT, 128, 2 * I)
            w2t[le] = w2[e].T.astype(FP8).reshape(IT, 128, D)
        xgr = xgT.reshape(U, 2, 128, E_LOC * cap).transpose(0, 2, 1, 3)
        xge = np.stack([xgr[:, :, :, le * cap:(le + 1) * cap]
                        for le in range(E_LOC)])
        in_maps.append({
            "xg": np.ascontiguousarray(xge),
            "comb": np.ascontiguousarray(cg.reshape(E_LOC, MT, 128, 1)),
            "w13t": w13t,
            "w2t": w2t,
        })

    res = run_bass_kernel_spmd(nc, in_maps, list(range(N_CORES)))

    out = np.zeros((T, D), np.float32)
    for core in range(N_CORES):
        ye = np.asarray(res.results[core]["ye"], dtype=np.float32)
        for le in range(E_LOC):
            e = core * E_LOC + le
            ix = idx[e]
            out[ix] += ye[le, :len(ix)]
    return out
